# revision 63
# baseline (speedup 1.0000x reference)
"""Distributed Trainium2 (8 NeuronCores) kernel for the 3-node ConvGRU
message-passing network.

Strategy (memory-bound: the five big projection matrices dominate traffic):
  - td projections (the two largest matrices, 118 MB f32 each) run in
    fp8e4m3 with DoubleRow perf mode: weights stored [98, 32 k-pairs, 2,
    1184] (pair planes stride %16), maxpooled activations transposed into
    [98, ch-pair, 2, B] fp8 lhsT tiles -> 2x PE throughput and half the
    HBM traffic of bf16 (end-to-end error improves: td noise is strongly
    compressed by the gates). bu projections + everything else stay bf16
    (bu in fp8 fails the error gate); PSUM accumulates f32.
  - Weights tensor-sharded across the 8 cores by output feature, stored
    partition-major so each streaming DMA reads contiguous slabs.
  - Convs: 6 accumulating matmul passes per conv (3x K=112 pair-taps using
    an x+1-shifted copy of the input stacked on partitions 64:112, plus 3x
    K=48 single taps) instead of 9 passes of K=48.
  - Big matmuls: lhsT = transposed activations per (slab, ch) chunk
    (s-major, so the first half of each contraction only needs maxpool
    slab 0), rhs = streamed weight tiles; outputs evacuated to [B, O] sba,
    transposed in W-wide blocks, biased, and bounced to DRAM with output
    feature f living at matmul column (f%nj)*W + f//nj so the bounce DMA
    writes nj*B contiguous bytes per partition (one descriptor each).
    Two AllGathers per timestep: B(u)={td1(u),bu1(u)} -> cell1(u);
    A(u)={bu2(u),bu0(u+1),td0(u+1)} -> cell2(u), cell0(u+1). (A per-matrix
    split into 4 gathers/step measured slightly worse on HW.)
  - Latency hiding: bu0(u+1) (x-only input) fills the gather-B window;
    cell0(u+1) runs in its own comb2/rz2 buffers, deferred so it
    interleaves with td1(u+1)'s matmul stream (generator zip); maxpool
    x AND y passes are emitted inside the cell right behind each GRU
    update half (into per-node mpq buffers), so the next stream's slab-0
    transposes are never queued behind the second update half; slab-1
    transposes overlap the first half of each contraction via big_matmul's
    mid= hook; cell assembly is fused adds (h+td etc.) with the
    x+1-shifted block written directly from sources (no serial ACT shift),
    bu reloaded straight into comb[HID:CIN] (td added in place, subtracted
    back out for the cand conv), the bu-independent h-part emitted before
    the reload lands, and the first 3 conv chunks (rows <= 8) emitted
    right after the first assembly half; the first weight tile's DMA is
    split so the stream's first matmul starts sooner.
  - Queue discipline: weight streams on SP/HWDGE, bounce writes colocated
    with the collectives on the gpsimd/SWDGE queue, reloads alternate
    ACT/gpsimd queues so their DGE-config times overlap, maxpool/assembly
    on DVE. Reload channels are l-major interleaved across cores (device
    slot l*8+c = core c's l-th channel; td shards own torch h channels
    {c, 8+c, 16+c, 24+c} + bu {2c, 2c+1}), so one DMA per channel group
    covers all 8 cores: 2 DMAs per bu reload / 6 per td instead of 8.
  - fc1 is output-sharded (13 of 104 padded outputs per core, full
    contraction) with one tiny f32 partial AllGather at the end.

Measured: single-core TimelineSim 2.05 ms for the full t_end=10 run
(baseline 2.31 ms); 8-core HW rel err 8.1e-3 (gate 2e-2); HBM weight
traffic ~25.6 MB/core/step vs 39.8 baseline.

Self-contained: hardcodes all shapes; host-side numpy does the sharding,
permutation, bf16/fp8 conversion and final unshard.
"""
import sys
import numpy as np
import ml_dtypes

for _p in ("/opt/trn_rl_repo", "/opt/pypackages",
           "/root/.axon_site", "/root/.axon_site/_ro/trn_rl_repo",
           "/root/.axon_site/_ro/pypackages"):
    if _p not in sys.path:
        sys.path.append(_p)

import concourse.bass as bass
import concourse.bacc as bacc
import concourse.mybir as mybir
import concourse.tile as tile
from concourse import bass_utils

F32 = mybir.dt.float32
F32R = mybir.dt.float32r
BF16 = mybir.dt.bfloat16
F8 = mybir.dt.float8e4
AF = mybir.ActivationFunctionType
GDT = BF16                   # dtype of the gather path (bounce + reload)
NPBF = ml_dtypes.bfloat16
NP8 = ml_dtypes.float8_e4m3

NCORES = 8
B, T, C, H, W = 16, 8, 3, 14, 14
HID, IND, N = 32, 16, 3
CIN = IND + HID              # 48 conv input channels
YP = XP = 16                 # padded spatial
# conv valid output flat window (phys coords, (y*XP+x)*B): (1,1)..(14,14)
WSTART = (1 * XP + 1) * B
WLEN = ((14 * XP + 14) - (1 * XP + 1) + 1) * B    # 3552
FLAT = YP * XP * B           # 4096

KP = 98                      # partitions per feature chunk (7 y-rows x 14 x)
KH = 2 * HID                 # 64 chunks for hidden-sized contraction (6272)
KHP = KH // 2                # 32 physical chunks for fp8 DoubleRow (k-pairs)
KX = 2 * C                   # 6 chunks for x contraction (588)
O_TD = (IND + HID) * H * W   # 9408
O_BU = IND * H * W           # 3136
OTD8 = O_TD // NCORES        # 1176 = 6 channels
OBU8 = O_BU // NCORES        # 392  = 2 channels
NJ_TD = 10                   # o-blocks per td shard
NJ_BU = 4
WTD = 118                    # o-block width (transpose partitions); feature
WBU = 98                     # f sits at block j=f%nj, row p=f//nj, so the
OTD8P = WTD * NJ_TD          # bounce DMA writes nj*B contiguous bytes per
                             # partition (1180: 4 zero-pad cols for td)
OQ8 = 1184                   # fp8 DoubleRow pair-plane stride (%16 == 0)
GRP_TD = 8                   # weight K-chunks per DMA (td)
GRP_BU = 8
OFC = 13                     # fc1 output columns per core (8*13=104 >= 100)
KPAIR = 112                  # pair-tap conv K: 48 + 16 zero pad + 48 shifted

_CACHED = {}


# ---------------------------------------------------------------- graph ----
def build_graph(t_end=T + N - 1, debug_h=False, no_cc=False, split_cc=False):
    # split_cc: False (default) = one gather per round — measured best.
    # "A" = also gather bu2/bu0 separately under td0's stream (+225us on
    # HW); True = split both rounds (+125us). Extra collective launches
    # cost more than the overlap they buy on this hardware.
    sa = split_cc in (True, "A")
    sb = split_cc is True
    nc = bacc.Bacc(None, target_bir_lowering=False, debug=False,
                   num_devices=NCORES)

    dp = nc.declare_dram_parameter
    # streamed weight shards, partition-major [98, K, O/8] bf16
    # td weights fp8e4m3 DoubleRow-packed: [98, 32 k-pairs, 2, OQ8] — the
    # two logical k rows of a pair are separate planes (BIR wants the
    # rhs AP's second dim Num=2 with plane stride % 16 == 0)
    tw0 = dp("tw0", [KP, KHP, 2, OQ8], F8, isOutput=False)
    tw1 = dp("tw1", [KP, KHP, 2, OQ8], F8, isOutput=False)
    bw0 = dp("bw0", [KP, KX, OBU8], BF16, isOutput=False)
    bw1 = dp("bw1", [KP, KH, OBU8], BF16, isOutput=False)
    bw2 = dp("bw2", [KP, KH, OBU8], BF16, isOutput=False)
    # bias shards (o-chunk padded) f32
    tb0 = dp("tb0", [NJ_TD, 128], F32, isOutput=False)
    tb1 = dp("tb1", [NJ_TD, 128], F32, isOutput=False)
    bb0 = dp("bb0", [NJ_BU, 128], F32, isOutput=False)
    bb1 = dp("bb1", [NJ_BU, 128], F32, isOutput=False)
    bb2 = dp("bb2", [NJ_BU, 128], F32, isOutput=False)
    # pre-transposed input x: [t, 98, k, B] bf16 (partition-major)
    xt_in = dp("xt", [T, KP, KX, B], BF16, isOutput=False)
    # conv weights: pair-tap packed [node, dy, 96, co] + single-tap [.., 48, co]
    wg2_in = dp("wg2", [N, 3, KPAIR, 2 * HID], BF16, isOutput=False)
    wg1_in = dp("wg1", [N, 3, CIN, 2 * HID], BF16, isOutput=False)
    wc2_in = dp("wc2", [N, 3, KPAIR, HID], BF16, isOutput=False)
    wc1_in = dp("wc1", [N, 3, CIN, HID], BF16, isOutput=False)
    bg_in = dp("bg", [N, 2 * HID], F32, isOutput=False)
    bc_in = dp("bc", [N, HID], F32, isOutput=False)
    # fc (fc1 output-sharded: this core's OFC output columns)
    fc1_in = dp("fc1t", [KP, KH, OFC], BF16, isOutput=False)
    fc1b_in = dp("fc1b", [100, 1], F32, isOutput=False)
    fc2_in = dp("fc2t", [100, 10], F32, isOutput=False)
    fc2b_in = dp("fc2b", [10, 1], F32, isOutput=False)
    ident_in = dp("ident", [32, 32], BF16, isOutput=False)
    out_ext = dp("out", [10, B], F32, isOutput=True)
    dbg_ext = dp("dbg", [N, HID, 14, 14, B], F32, isOutput=True) if debug_h else None

    from contextlib import ExitStack
    with tile.TileContext(nc) as tc, ExitStack() as ctx:
        consts = ctx.enter_context(tc.tile_pool(name="consts", bufs=1))
        wtd_pool = ctx.enter_context(tc.tile_pool(name="wtd", bufs=2))
        wbu_pool = ctx.enter_context(tc.tile_pool(name="wbu", bufs=2))
        mpt_pool = ctx.enter_context(tc.tile_pool(name="mpt", bufs=4))
        pst_pool = ctx.enter_context(tc.tile_pool(name="pst", bufs=2, space="PSUM"))
        acc_pool = ctx.enter_context(tc.tile_pool(name="accp", bufs=1, space="PSUM"))
        conv_pool = ctx.enter_context(tc.tile_pool(name="convp", bufs=2, space="PSUM"))
        sbacc_pool = ctx.enter_context(tc.tile_pool(name="sbacc", bufs=1))
        outt_pool = ctx.enter_context(tc.tile_pool(name="outt", bufs=2))
        dram = ctx.enter_context(tc.tile_pool(name="dram", bufs=1, space="DRAM"))

        # ---------------- constants ----------------
        ident = consts.tile([32, 32], BF16)
        nc.sync.dma_start(ident[:], ident_in[:])
        wg2_sb = consts.tile([KPAIR, N, 3, 2 * HID], BF16)
        nc.sync.dma_start(wg2_sb[:], wg2_in[:].rearrange("n s c o -> c n s o"))
        wg1_sb = consts.tile([CIN, N, 3, 2 * HID], BF16)
        nc.sync.dma_start(wg1_sb[:], wg1_in[:].rearrange("n s c o -> c n s o"))
        wc2_sb = consts.tile([KPAIR, N, 3, HID], BF16)
        nc.sync.dma_start(wc2_sb[:], wc2_in[:].rearrange("n s c o -> c n s o"))
        wc1_sb = consts.tile([CIN, N, 3, HID], BF16)
        nc.sync.dma_start(wc1_sb[:], wc1_in[:].rearrange("n s c o -> c n s o"))
        bg_sb = consts.tile([2 * HID, N], F32)
        nc.sync.dma_start(bg_sb[:], bg_in[:].rearrange("n o -> o n"))
        bc_sb = consts.tile([HID, N], F32)
        nc.sync.dma_start(bc_sb[:], bc_in[:].rearrange("n o -> o n"))
        tb0_sb = consts.tile([128, NJ_TD], F32)
        nc.sync.dma_start(tb0_sb[:], tb0[:].rearrange("j p -> p j"))
        tb1_sb = consts.tile([128, NJ_TD], F32)
        nc.sync.dma_start(tb1_sb[:], tb1[:].rearrange("j p -> p j"))
        bb0_sb = consts.tile([128, NJ_BU], F32)
        nc.sync.dma_start(bb0_sb[:], bb0[:].rearrange("j p -> p j"))
        bb1_sb = consts.tile([128, NJ_BU], F32)
        nc.sync.dma_start(bb1_sb[:], bb1[:].rearrange("j p -> p j"))
        bb2_sb = consts.tile([128, NJ_BU], F32)
        nc.sync.dma_start(bb2_sb[:], bb2[:].rearrange("j p -> p j"))
        fc2_sb = consts.tile([100, 10], F32)
        nc.sync.dma_start(fc2_sb[:], fc2_in[:])
        fc1b_sb = consts.tile([100, 1], F32)
        nc.sync.dma_start(fc1b_sb[:], fc1b_in[:])
        fc2b_sb = consts.tile([10, 1], F32)
        nc.sync.dma_start(fc2b_sb[:], fc2b_in[:])
        xt_all = consts.tile([KP, T, KX, B], BF16)
        nc.sync.dma_start(xt_all[:], xt_in[:].rearrange("t p k b -> p t k b"))

        # ------------- dedicated activation tensors (shared/aliased) -------
        h = [consts.tile([HID, YP, XP, B], BF16, name=f"h{i}", tag=f"h{i}")
             for i in range(N)]
        # conv input, pair-tap stacked: [0:48]=comb, [64:112]=comb shifted
        # +1 x; [48:64] stays zero (partition bases must be 0/32/64/96).
        # comb2/rz2 let cell0 interleave with cell2 inside round A.
        comb = consts.tile([KPAIR, YP, XP, B], BF16)
        comb2 = consts.tile([KPAIR, YP, XP, B], BF16)
        rz = consts.tile([2 * HID, YP, XP, B], BF16)  # gates; [0:HID] doubles
        #   as cand / maxpool output / relu buffer
        rz2 = consts.tile([2 * HID, YP, XP, B], BF16)
        td_buf = [consts.tile([CIN, YP, XP, B], GDT, name=f"td{i}", tag=f"td{i}")
                  for i in range(2)]
        for tt in h + td_buf + [rz, rz2, comb, comb2]:
            nc.vector.memset(tt[:], 0.0)

        # maxpool scratch: per-cell x-pass scratch (tmq2 isolates cell2,
        # which interleaves with cell0) and per-node maxpool output, so the
        # y-passes can be emitted inside the cell right after each update
        # half without clobbering a not-yet-transposed earlier maxpool.
        tmq = consts.tile([HID, YP, XP, B], BF16, name="tmq", tag="tmq")
        tmq2 = consts.tile([HID, YP, XP, B], BF16, name="tmq2", tag="tmq2")
        mpq = [consts.tile([HID, YP, XP, B], BF16, name=f"mpq{i}",
                           tag=f"mpq{i}") for i in range(N)]

        # ---------------- helpers ----------------
        def mp_x_pass(src, tq=None, y0=1, y1=15, eng=None):
            tq = tmq if tq is None else tq
            eng = nc.vector if eng is None else eng
            eng.tensor_max(tq[0:HID, y0:y1, 2:14, :], src[0:HID, y0:y1, 1:13, :], src[0:HID, y0:y1, 2:14, :])
            eng.tensor_max(tq[0:HID, y0:y1, 2:14, :], tq[0:HID, y0:y1, 2:14, :], src[0:HID, y0:y1, 3:15, :])
            eng.tensor_max(tq[0:HID, y0:y1, 1:2, :], src[0:HID, y0:y1, 1:2, :], src[0:HID, y0:y1, 2:3, :])
            eng.tensor_max(tq[0:HID, y0:y1, 14:15, :], src[0:HID, y0:y1, 13:14, :], src[0:HID, y0:y1, 14:15, :])

        def mp_y_pass(s, tq, mp):
            if s == 0:
                nc.vector.tensor_max(mp[0:HID, 2:8, 1:15, :], tq[0:HID, 1:7, 1:15, :], tq[0:HID, 2:8, 1:15, :])
                nc.vector.tensor_max(mp[0:HID, 2:8, 1:15, :], mp[0:HID, 2:8, 1:15, :], tq[0:HID, 3:9, 1:15, :])
                nc.vector.tensor_max(mp[0:HID, 1:2, 1:15, :], tq[0:HID, 1:2, 1:15, :], tq[0:HID, 2:3, 1:15, :])
            else:
                nc.vector.tensor_max(mp[0:HID, 8:14, 1:15, :], tq[0:HID, 7:13, 1:15, :], tq[0:HID, 8:14, 1:15, :])
                nc.vector.tensor_max(mp[0:HID, 8:14, 1:15, :], mp[0:HID, 8:14, 1:15, :], tq[0:HID, 9:15, 1:15, :])
                nc.vector.tensor_max(mp[0:HID, 14:15, 1:15, :], tq[0:HID, 13:14, 1:15, :], tq[0:HID, 14:15, 1:15, :])

        def transpose_slab(src, s, mt, mt8=None):
            """slab s of src[0:HID] (y rows 1+7s .. 8+7s) -> mt [98, HID, B]
            (bf16) and/or mt8 [98, HID/2, B, 2] (fp8, channel-pair
            interleaved for DoubleRow lhsT). PE transpose needs a single-
            free-dim input, so first repack the (y, x)-strided valid slice
            contiguously per batch."""
            y0 = 1 + 7 * s
            stg = mpt_pool.tile([HID, B, KP], BF16, tag="stg", name="stg", bufs=2)
            nc.vector.tensor_copy(
                stg[:].rearrange("c b (y x) -> c y x b", y=7, x=14),
                src[0:HID, y0:y0 + 7, 1:15, :])
            for b in range(B):
                pt = pst_pool.tile([128, HID], BF16, tag="psT", name="ptt")
                nc.tensor.transpose(pt[:KP, 0:HID], stg[:, b, :].opt(),
                                    ident[0:HID, 0:HID])
                if mt is not None:
                    nc.scalar.activation(mt[:, 0:HID, b], pt[:KP, 0:HID],
                                         AF.Copy)
                if mt8 is not None:
                    nc.scalar.activation(
                        mt8[:, :, :, b],
                        pt[:KP, 0:HID].rearrange("p (cc i) -> p cc i", i=2),
                        AF.Copy)

        def maxpool_transpose(node, f16=True, f8=False):
            """mpq[node] (maxpool of h[node]; x/y passes were emitted inside
            the cell, right behind each update half) -> (m16, m8,
            finish_slab1). Slab 0 transposes emit immediately; the returned
            callback emits slab 1 and is passed to big_matmul's mid= hook so
            the first (s-major) half of the contraction overlaps it."""
            out = ([mpt_pool.tile([KP, HID, B], BF16, tag="mpt",
                                  name=f"mpt{s}") for s in range(2)]
                   if f16 else None)
            out8 = ([mpt_pool.tile([KP, HID // 2, 2, B], F8, tag="mpt8",
                                   name=f"mpt8{s}") for s in range(2)]
                    if f8 else None)
            transpose_slab(mpq[node], 0, out and out[0], out8 and out8[0])

            def finish():
                transpose_slab(mpq[node], 1, out and out[1],
                               out8 and out8[1])
            return out, out8, finish

        def transpose_feat(src):
            out = [mpt_pool.tile([KP, HID, B], BF16, tag="mpt", name=f"mpt{s}")
                   for s in range(2)]
            for s in range(2):
                transpose_slab(src, s, out[s])
            return out

        def big_matmul_gen(nk, o8, W_, nj, lhsT_of, w_dram, grp, bias_sb,
                           agin, row_off, mid=None, dr=False):
            """Streamed o-sharded matmul: out.T[o8, B] = W_shard @ act (+bias),
            written into agin[row_off : row_off+o8, :]. Device feature order:
            feature f lives at matmul column (f%nj)*W_ + f//nj, so agin row f
            sits at (partition f//nj, j=f%nj) of outT and the bounce DMA is
            one nj*B-byte descriptor per partition. k order is s-major
            (chunks 0..nk/2-1 use activation slab 0)."""
            nslice = (o8 + 511) // 512
            pacc = acc_pool.tile([B, 512 * nslice], F32,
                                 tag=("acc" if nslice > 1 else "accbu"),
                                 name="pacc")
            for g in range(0, nk, grp):
                if mid is not None and g == nk // 2:
                    mid()
                pool = wtd_pool if o8 == OTD8P else wbu_pool
                if dr:
                    wt = pool.tile([KP, grp, 2, OQ8], F8, tag="w", name="wt")
                    wsrc = w_dram[:, g:g + grp, :, :]
                else:
                    wt = pool.tile([KP, grp, o8], BF16, tag="w", name="wt")
                    wsrc = w_dram[:, g:g + grp, :]
                if g == 0 and grp >= 4:
                    # split the first tile's DMA so matmul 0 starts sooner
                    h1 = grp // 4
                    nc.sync.dma_start(wt[:, 0:h1], wsrc[:, 0:h1])
                    nc.sync.dma_start(wt[:, h1:grp], wsrc[:, h1:grp])
                else:
                    nc.sync.dma_start(wt[:], wsrc)
                for j in range(grp):
                    k = g + j
                    for sl in range(nslice):
                        o0 = sl * 512
                        ln = min(512, o8 - o0)
                        if dr:
                            nc.tensor.matmul(
                                pacc[:, o0:o0 + ln],
                                lhsT_of(k),
                                wt[:, j, :, o0:o0 + ln],
                                start=(k == 0), stop=(k == nk - 1),
                                perf_mode=mybir.MatmulPerfMode.DoubleRow,
                            )
                        else:
                            nc.tensor.matmul(
                                pacc[:, o0:o0 + ln],
                                lhsT_of(k).opt(),
                                wt[:, j, o0:o0 + ln].opt(),
                                start=(k == 0), stop=(k == nk - 1),
                            )
                yield
            sba = sbacc_pool.tile([B, o8], BF16, tag="sba", name="sba", bufs=2)
            for sl in range(nslice):
                o0 = sl * 512
                ln = min(512, o8 - o0)
                nc.scalar.activation(sba[:, o0:o0 + ln], pacc[:, o0:o0 + ln],
                                     AF.Copy)
            outT = outt_pool.tile([128, nj, B], GDT, tag="outT", name="outT")
            for jj in range(nj):
                pt = pst_pool.tile([128, HID], BF16, tag="psT", name="pt2")
                nc.tensor.transpose(pt[:W_, 0:B], sba[:, jj * W_: (jj + 1) * W_],
                                    ident[0:B, 0:B])
                nc.scalar.activation(outT[:W_, jj, :], pt[:W_, 0:B], AF.Identity,
                                     bias=bias_sb[0:W_, jj:jj + 1])
            nc.gpsimd.dma_start(
                agin[row_off: row_off + o8, :].rearrange(
                    "(p j) b -> p j b", j=nj),
                outT[0:W_, :, :])

        def big_matmul(*a, **kw):
            for _ in big_matmul_gen(*a, **kw):
                pass

        def do_gather(agin, agout):
            if no_cc:
                # sim-only stand-in for the AllGather: flat views (one
                # descriptor per copy) + log2 doubling
                af = agin[:].rearrange("r b -> (r b)")
                of = agout[:].rearrange("c r b -> c (r b)")
                nc.gpsimd.dma_start(of[0], af)
                nc.gpsimd.dma_start(of[1], of[0])
                nc.gpsimd.dma_start(of[2:4], of[0:2])
                nc.gpsimd.dma_start(of[4:8], of[0:4])
            else:
                nc.gpsimd.collective_compute(
                    "AllGather", mybir.AluOpType.bypass,
                    replica_groups=[list(range(NCORES))],
                    ins=[agin.opt()], outs=[agout.opt()])

        def reload(buf, agout, row_off, nch_l, pbase=0):
            """agout [8, rows, B] o-major (bf16) -> buf partitions
            [pbase : pbase+8*nch_l], interior 14x14. Channels are l-major
            interleaved (device slot l*8+c holds core c's l-th channel), so
            ONE DMA per channel group covers all 8 cores: dst partitions
            [l*8, (l+1)*8) are contiguous and the core stride in agout is
            uniform. DMAs alternate ACT/gpsimd queues."""
            for l in range(nch_l):
                srcv = agout[:, row_off + l * 196: row_off + (l + 1) * 196,
                             :].rearrange("c (y x) b -> c y x b", y=14, x=14)
                eng = nc.scalar if l % 2 == 0 else nc.gpsimd
                eng.dma_start(
                    buf[pbase + 8 * l: pbase + 8 * (l + 1), 1:15, 1:15, :],
                    srcv)

        def conv6_gen(w2_of, w1_of, nco, bias_ap, out_t, act_fn, cmb):
            inp_f = cmb[:].rearrange("c y x b -> c (y x b)")
            out_f = out_t.rearrange("c y x b -> c (y x b)")
            q = 0
            while q < WLEN:
                ln = min(512, WLEN - q)
                pc = conv_pool.tile([nco, 512], F32, tag="conv", name="pc")
                for i, dy in enumerate((-1, 0, 1)):
                    offp = (dy * XP - 1) * B     # pair taps (dy,-1)+(dy,0)
                    nc.tensor.matmul(
                        pc[:, 0:ln],
                        w2_of(i).opt(),
                        inp_f[0:KPAIR, WSTART + q + offp: WSTART + q + offp + ln],
                        start=(i == 0), stop=False,
                    )
                    offs = (dy * XP + 1) * B     # single tap (dy,+1)
                    nc.tensor.matmul(
                        pc[:, 0:ln],
                        w1_of(i).opt(),
                        inp_f[0:CIN, WSTART + q + offs: WSTART + q + offs + ln],
                        start=False, stop=(i == 2),
                    )
                nc.scalar.activation(out_f[:, WSTART + q: WSTART + q + ln],
                                     pc[:, 0:ln], act_fn, bias=bias_ap)
                q += ln
                yield

        def asm_gate_h(node, td_t, y0, y1, cmb):
            """h-only part of the gate-conv assembly (independent of the
            gathered bu, so it can be emitted before the bu reload lands):
            cmb[0:HID]=h(+td) plus the x+1-shifted copy at 64:96."""
            hh = h[node]
            cf = cmb[:].rearrange("c y x b -> c (y x b)")
            hf = hh[:].rearrange("c y x b -> c (y x b)")
            f0 = max(0, y0 * XP * B - B)
            f1 = min(FLAT - B, y1 * XP * B - B)
            if td_t is not None:
                tf = td_t[:].rearrange("c y x b -> c (y x b)")
                nc.vector.tensor_add(cmb[0:HID, y0:y1, :, :],
                                     hh[:, y0:y1, :, :],
                                     td_t[0:HID, y0:y1, :, :])
                nc.vector.tensor_add(cf[64:64 + HID, f0:f1],
                                     hf[0:HID, f0 + B:f1 + B],
                                     tf[0:HID, f0 + B:f1 + B])
            else:
                nc.vector.tensor_copy(cmb[0:HID, y0:y1, :, :],
                                      hh[:, y0:y1, :, :])
                nc.vector.tensor_copy(cf[64:64 + HID, f0:f1],
                                      hf[0:HID, f0 + B:f1 + B])

        def asm_gate_bu(node, td_t, y0, y1, cmb):
            """bu part: cmb[HID:CIN]=bu(+td) (bu lands there via reload)
            plus the shifted copy at 96:112."""
            cf = cmb[:].rearrange("c y x b -> c (y x b)")
            f0 = max(0, y0 * XP * B - B)
            f1 = min(FLAT - B, y1 * XP * B - B)
            if td_t is not None:
                nc.vector.tensor_add(cmb[HID:CIN, y0:y1, :, :],
                                     cmb[HID:CIN, y0:y1, :, :],
                                     td_t[HID:CIN, y0:y1, :, :])
            nc.vector.tensor_copy(cf[64 + HID:KPAIR, f0:f1],
                                  cf[HID:CIN, f0 + B:f1 + B])

        def asm_cand(node, td_t, y0, y1, cmb, rzt):
            """cmb -> cand-conv input rows [y0,y1): [r*h | bu] + shifts.
            cmb[HID:CIN] holds bu+td; subtract td back out to recover bu."""
            hh = h[node]
            cf = cmb[:].rearrange("c y x b -> c (y x b)")
            rf = rzt[:].rearrange("c y x b -> c (y x b)")
            hf = hh[:].rearrange("c y x b -> c (y x b)")
            f0 = max(0, y0 * XP * B - B)
            f1 = min(FLAT - B, y1 * XP * B - B)
            nc.vector.tensor_mul(cmb[0:HID, y0:y1, :, :],
                                 rzt[0:HID, y0:y1, :, :], hh[:, y0:y1, :, :])
            nc.vector.tensor_mul(cf[64:64 + HID, f0:f1],
                                 rf[0:HID, f0 + B:f1 + B],
                                 hf[0:HID, f0 + B:f1 + B])
            if td_t is not None:
                nc.vector.tensor_sub(cmb[HID:CIN, y0:y1, :, :],
                                     cmb[HID:CIN, y0:y1, :, :],
                                     td_t[HID:CIN, y0:y1, :, :])
                nc.vector.tensor_copy(cf[64 + HID:KPAIR, f0:f1],
                                      cf[HID:CIN, f0 + B:f1 + B])

        def cell_gen(node, td_t, xq, cmb, rzt):
            """GRU cell update of h[node]; bu arrives in cmb[HID:CIN] via
            reload. The first 3 conv chunks (rows <= 8) are emitted right
            after the first assembly half so PE starts while the second half
            assembles; generator yields let an interleaved partner (another
            cell or a matmul stream) fill the gaps. The maxpool x AND y
            passes of the fresh h are emitted right behind each update half
            into mpq[node], so the next round's slab-0 transposes aren't
            queued behind the second update half."""
            hh = h[node]
            asm_gate_h(node, td_t, 0, 9, cmb)
            asm_gate_h(node, td_t, 9, 16, cmb)
            asm_gate_bu(node, td_t, 0, 9, cmb)
            yield
            gg = conv6_gen(lambda i: wg2_sb[:, node, i, :],
                           lambda i: wg1_sb[:, node, i, :],
                           2 * HID, bg_sb[:, node:node + 1], rzt[:],
                           AF.Sigmoid, cmb)
            for _ in range(3):
                next(gg)
                yield
            asm_gate_bu(node, td_t, 9, 16, cmb)
            yield
            yield from gg
            asm_cand(node, td_t, 0, 9, cmb, rzt)
            yield
            cg = conv6_gen(lambda i: wc2_sb[:, node, i, :],
                           lambda i: wc1_sb[:, node, i, :],
                           HID, bc_sb[:, node:node + 1],
                           rzt[0:HID, :, :, :], AF.Tanh, cmb)
            for _ in range(3):
                next(cg)
                yield
            asm_cand(node, td_t, 9, 16, cmb, rzt)
            yield
            yield from cg
            for s, (y0, y1) in enumerate(((1, 9), (9, 15))):
                hv = hh[:, y0:y1, 1:15, :]
                cv = rzt[0:HID, y0:y1, 1:15, :]
                # z lives at base partition 32; DVE tensor-tensor ops need
                # equal base partitions, so stage it at base 0 in cmb.
                zc = cmb[0:HID, y0:y1, 1:15, :]
                nc.vector.tensor_copy(zc, rzt[HID:2 * HID, y0:y1, 1:15, :])
                nc.vector.tensor_sub(cv, cv, hv)
                nc.vector.tensor_mul(cv, cv, zc)
                nc.vector.tensor_add(hv, hv, cv)
                mp_x_pass(hh, xq, y0, y1)
                mp_y_pass(s, xq, mpq[node])
                yield

        def cell(node, td_t, xq, cmb, rzt):
            for _ in cell_gen(node, td_t, xq, cmb, rzt):
                pass

        def zip2(ga, gb, ratio=2):
            """Interleave two generators, giving `ga` (the critical-path
            cell) `ratio` steps per `gb` step."""
            while ga is not None or gb is not None:
                for _ in range(ratio):
                    if ga is not None:
                        try:
                            next(ga)
                        except StopIteration:
                            ga = None
                if gb is not None:
                    try:
                        next(gb)
                    except StopIteration:
                        gb = None

        # ------------- round schedule: 2 collectives per timestep -------------
        # Round u (u = timestep of cell1/cell2):
        #   B(u): gather {td1(u) [u>=2], bu1(u)} -> cell1(u)
        #   A(u): gather {bu2(u) [u>=1], bu0(u+1) [u+1<T], td0(u+1) [2<=u+1<T]}
        #         -> cell2(u) [u>=1], cell0(u+1) [u+1<T]
        # bu2(u) and td0(u+1) share mp(h1@u) (one maxpool+transpose).
        # cell0(u+1) is returned as a pending generator and interleaved with
        # td1(u+1)'s matmul stream at the start of the next round_B.
        def lam(m):
            return lambda k, mm=m: mm[k // HID][:, (k % HID), :]

        def lam8(m8):
            # physical chunk kk -> [98, 2, B] fp8 lhsT: channel pair
            # (2cc, 2cc+1) of slab kk//16 as two B-column planes
            return lambda kk, mm=m8: mm[kk // (HID // 2)][
                :, kk % (HID // 2), :, :]

        def mk_ag(rows, name):
            agin = dram.tile([rows, B], GDT, name=f"agin{name}",
                             tag=f"agin{name}")
            agout = dram.tile([NCORES, rows, B], GDT, name=f"agout{name}",
                              tag=f"agout{name}",
                              addr_space="Local" if no_cc else "Shared")
            return agin, agout

        # Per-matrix gathers: the first collective of each round launches as
        # soon as its matrix's stream ends and completes under the next
        # stream (round B: td1-gather under bu1's stream; round A: bu2/bu0-
        # gather under td0's stream, and td0's own gather under cell2's
        # convs). Only bu1's small gather is exposed per step.
        def round_A_pre(u):
            hbu2 = 1 <= u < t_end
            hbu0 = u + 1 < min(T, t_end)
            htd0 = 2 <= u + 1 < min(T, t_end)
            if not (hbu2 or hbu0 or htd0):
                return None
            st = dict(hbu2=hbu2, hbu0=hbu0, htd0=htd0,
                      ro_bu0=(OBU8 if hbu2 else 0))
            rows1 = st["ro_bu0"] + (OBU8 if hbu0 else 0)
            st["rows1"] = rows1
            if sa:
                if rows1:
                    st["ag1"] = mk_ag(rows1, f"A1_{u}")
                if htd0:
                    st["ag2"] = mk_ag(OTD8P, f"A2_{u}")
            else:
                # one gather for the whole round: td0 rows follow bu2/bu0
                ag = mk_ag(rows1 + (OTD8P if htd0 else 0), f"A1_{u}")
                st["ag1"] = ag
                st["ag2"] = (ag[0], ag[1], rows1) if htd0 else None
            if hbu0:
                # bu0 depends only on x: its matmuls slot into the PE-idle
                # window while round_B's bu1 gather + reload are in flight.
                big_matmul(KX, OBU8, WBU, NJ_BU,
                           lambda k: xt_all[:, u + 1, k, :], bw0, KX,
                           bb0_sb, st["ag1"][0], st["ro_bu0"])
            return st

        def round_A_rest(u, st):
            if st is None:
                return None
            hbu2, hbu0, htd0 = st["hbu2"], st["hbu0"], st["htd0"]
            mid1 = None
            if hbu2 or htd0:
                m1, m1_8, mid1 = maxpool_transpose(1, f16=hbu2, f8=htd0)
            if hbu2:
                big_matmul(KH, OBU8, WBU, NJ_BU, lam(m1), bw2, GRP_BU,
                           bb2_sb, st["ag1"][0], 0, mid=mid1)
                mid1 = None
            if sa and st["rows1"]:
                do_gather(*st["ag1"])
            if htd0:
                ro_td = st["ag2"][2] if not sa else 0
                big_matmul(KHP, OTD8P, WTD, NJ_TD, lam8(m1_8), tw0, GRP_TD,
                           tb0_sb, st["ag2"][0], ro_td, mid=mid1, dr=True)
                mid1 = None
            if mid1 is not None:
                mid1()
            if not sa:
                do_gather(st["ag1"][0], st["ag1"][1])
            if hbu2:
                reload(comb, st["ag1"][1], 0, IND // NCORES, pbase=HID)
            if hbu0:
                reload(comb2, st["ag1"][1], st["ro_bu0"], IND // NCORES,
                       pbase=HID)
            if hbu2:
                cell(2, None, tmq2, comb, rz)
            if htd0:
                if sa:
                    do_gather(*st["ag2"])
                reload(td_buf[0], st["ag2"][1], st["ag2"][2]
                       if not sa else 0, CIN // NCORES)
            # cell0(u+1) runs in comb2/rz2, deferred so its elementwise work
            # interleaves with round_B(u+1)'s td1 matmul stream.
            if hbu0:
                return cell_gen(0, td_buf[0] if htd0 else None, tmq, comb2,
                                rz2)
            return None

        def round_B(u, pend_cell0):
            htd1 = u >= 2
            if sb:
                ag1 = mk_ag(OTD8P, f"B1_{u}") if htd1 else None
                ag2 = mk_ag(OBU8, f"B2_{u}")
                ro_bu1 = 0
            else:
                ag2 = mk_ag((OTD8P if htd1 else 0) + OBU8, f"B1_{u}")
                ag1 = ag2 if htd1 else None
                ro_bu1 = OTD8P if htd1 else 0
            if htd1:
                _, m2_8, mid2 = maxpool_transpose(2, f16=False, f8=True)
                td1_gen = big_matmul_gen(KHP, OTD8P, WTD, NJ_TD, lam8(m2_8),
                                         tw1, GRP_TD, tb1_sb, ag1[0], 0,
                                         mid=mid2, dr=True)
            else:
                td1_gen = None
            zip2(pend_cell0, td1_gen)
            if sb and htd1:
                do_gather(*ag1)
                reload(td_buf[1], ag1[1], 0, CIN // NCORES)
            m0, _, mid0 = maxpool_transpose(0)
            big_matmul(KH, OBU8, WBU, NJ_BU, lam(m0), bw1, GRP_BU,
                       bb1_sb, ag2[0], ro_bu1, mid=mid0)
            do_gather(ag2[0], ag2[1])
            stA = round_A_pre(u)   # bu0(u+1) fills the gather window
            if not sb and htd1:
                reload(td_buf[1], ag1[1], 0, CIN // NCORES)
            reload(comb, ag2[1], ro_bu1, IND // NCORES, pbase=HID)
            cell(1, td_buf[1] if htd1 else None, tmq, comb, rz)
            return stA

        pend = round_A_rest(-1, round_A_pre(-1))  # bootstrap: bu0(0)->cell0(0)
        for u in range(t_end):
            if u >= 1:
                stA = round_B(u, pend)
            else:
                zip2(pend, None)
                stA = round_A_pre(u)
            pend = round_A_rest(u, stA)
        zip2(pend, None)

        if debug_h:
            for i in range(N):
                nc.gpsimd.dma_start(dbg_ext[i], h[i][:, 1:15, 1:15, :])
        # -------- final FC head (fc1 output-sharded + partial gather) --------
        # Each core computes its OFC of the 104 (padded) fc1 outputs with the
        # full contraction, then one tiny AllGather assembles p1.
        nc.scalar.activation(rz[0:HID, :, :, :], h[2][:], AF.Relu)
        pT = transpose_feat(rz)
        pfc = acc_pool.tile([OFC, 16], F32, tag="acc", name="pfc")
        for g in range(0, KH, 8):
            wf = wtd_pool.tile([KP, 8, OFC], BF16, tag="w", name="wf")
            nc.sync.dma_start(wf[:], fc1_in[:, g:g + 8, :])
            for j in range(8):
                k = g + j
                nc.tensor.matmul(pfc[:], wf[:, j, :].opt(),
                                 pT[k // HID][:, (k % HID), :].opt(),
                                 start=(k == 0), stop=(k == KH - 1))
        pfs = sbacc_pool.tile([OFC, 16], F32, tag="pfs", name="pfs")
        nc.scalar.activation(pfs[:], pfc[:], AF.Copy)
        aginF = dram.tile([OFC, 16], F32, name="aginF", tag="aginF")
        nc.gpsimd.dma_start(aginF[:], pfs[:])
        agoutF = dram.tile([NCORES, OFC, 16], F32, name="agoutF", tag="agoutF",
                           addr_space="Local" if no_cc else "Shared")
        do_gather(aginF, agoutF)
        p1r = sbacc_pool.tile([NCORES * OFC, 16], F32, tag="p1r", name="p1r")
        nc.gpsimd.dma_start(p1r[:], agoutF[:].rearrange("c o b -> (c o) b"))
        p1 = sbacc_pool.tile([100, 16], F32, tag="p1", name="p1")
        nc.scalar.activation(p1[:], p1r[0:100, :], AF.Relu,
                             bias=fc1b_sb[:])
        pf2 = acc_pool.tile([128, HID], F32, tag="acc", name="pf2")
        nc.tensor.matmul(pf2[0:10, 0:16], fc2_sb[:], p1[:],
                         start=True, stop=True)
        osb = sbacc_pool.tile([10, 16], F32, tag="osb", name="osb")
        nc.scalar.activation(osb[:], pf2[0:10, 0:16], AF.Identity,
                             bias=fc2b_sb[:])
        nc.gpsimd.dma_start(out_ext[:], osb[:])

    nc.finalize()
    return nc


# ---------------------------------------------------------------- host ----
def _feat_perm(nch):
    """Device feature order (ch, s, p) -> torch flat feature index."""
    perm = np.zeros((nch * 2, KP), np.int64)
    for ch in range(nch):
        for s in range(2):
            k = s * nch + ch
            p = np.arange(KP)
            y = s * 7 + p // 14
            x = p % 14
            perm[k] = ch * 196 + y * 14 + x
    return perm


def _shard_w(wmat, nch_in, o8, W_, nj):
    """wmat (O, K) torch-order -> per-core [98, nk, W_*nj] bf16 shards.
    Device column j*W_ + p holds feature f = p*nj + j (zero-padded), so on
    device the bounce DMA writes agin rows partition-major."""
    perm = _feat_perm(nch_in)
    wt = wmat.T[perm.reshape(-1)].reshape(perm.shape[0], KP,
                                          wmat.shape[0]).astype(NPBF)
    o8p = W_ * nj
    c_idx = np.arange(o8p)
    f = (c_idx % W_) * nj + c_idx // W_
    valid = f < o8
    out = []
    for c in range(NCORES):
        blk = wt[:, :, c * o8:(c + 1) * o8]
        padded = np.zeros((wt.shape[0], KP, o8p), NPBF)
        padded[:, :, valid] = blk[:, :, f[valid]]
        out.append(np.ascontiguousarray(padded.transpose(1, 0, 2)))
    return out


def _shard_w8(wmat, o8, W_, nj):
    """td wmat (O, 6272) torch-order -> per-core [98, 32, 2*W_*nj] fp8e4m3
    DoubleRow shards: physical chunk kk = s*16+cc holds logical k-chunks
    (s*32+2cc, s*32+2cc+1) with their values interleaved along the output
    dim; output features permuted/padded as in _shard_w."""
    perm = _feat_perm(HID)
    wt = wmat.T[perm.reshape(-1)].reshape(KH, KP,
                                          wmat.shape[0]).astype(np.float32)
    o8p = W_ * nj
    c_idx = np.arange(o8p)
    f = (c_idx % W_) * nj + c_idx // W_
    valid = f < o8
    out = []
    for c in range(NCORES):
        blk = wt[:, :, c * o8:(c + 1) * o8]
        padded = np.zeros((KH, KP, o8p), np.float32)
        padded[:, :, valid] = blk[:, :, f[valid]]
        w8 = np.zeros((KP, KHP, 2, OQ8), NP8)
        for s in range(2):
            for cc in range(HID // 2):
                kk = s * (HID // 2) + cc
                k0 = s * HID + 2 * cc
                w8[:, kk, 0, :o8p] = padded[k0].astype(NP8)
                w8[:, kk, 1, :o8p] = padded[k0 + 1].astype(NP8)
        out.append(np.ascontiguousarray(w8))
    return out


def _pad_bias(bvec, o8, W_, nj):
    out = []
    p = np.arange(W_)
    for c in range(NCORES):
        bp = np.zeros((nj, 128), np.float32)
        for j in range(nj):
            f = p * nj + j
            m = f < o8
            bp[j, p[m]] = bvec[c * o8 + f[m]]
        out.append(bp)
    return out


def prep_inputs(inputs):
    x = np.asarray(inputs["x"], np.float32)
    permx = _feat_perm(C)
    xt = np.zeros((T, KP, KX, B), NPBF)
    for t in range(T):
        flat = x[:, t].reshape(B, C * 196).T      # [588, B]
        xt[t] = flat[permx.reshape(-1)].reshape(KX, KP, B).transpose(1, 0, 2)

    # td outputs are reloaded straight into device channel order [h, bu],
    # l-major interleaved across cores (device slot l*8+c = core c's l-th
    # channel, so each reload DMA covers all 8 cores): core c's td block is
    # [torch h {c, 8+c, 16+c, 24+c} | torch bu {2c, 2c+1}]. The h side is
    # the identity on device partitions; the bu side lands at slot
    # q=lb*8+c holding torch bu 2c+lb.
    ci_out = np.zeros(CIN, np.int64)
    for d in range(CIN):
        c, l = d // 6, d % 6
        ci_out[d] = (IND + l * 8 + c) if l < 4 else (2 * c + l - 4)
    o_perm = (ci_out[:, None] * 196 + np.arange(196)[None, :]).reshape(-1)
    tw0 = _shard_w8(np.asarray(inputs["td_w0"], np.float32)[o_perm],
                    OTD8, WTD, NJ_TD)
    tw1 = _shard_w8(np.asarray(inputs["td_w1"], np.float32)[o_perm],
                    OTD8, WTD, NJ_TD)
    bw0 = _shard_w(np.asarray(inputs["bu_w0"], np.float32), C, OBU8, WBU,
                   NJ_BU)
    bw1 = _shard_w(np.asarray(inputs["bu_w1"], np.float32), HID, OBU8, WBU,
                   NJ_BU)
    bw2 = _shard_w(np.asarray(inputs["bu_w2"], np.float32), HID, OBU8, WBU,
                   NJ_BU)
    tb0 = _pad_bias(np.asarray(inputs["td_b0"], np.float32)[o_perm], OTD8,
                    WTD, NJ_TD)
    tb1 = _pad_bias(np.asarray(inputs["td_b1"], np.float32)[o_perm], OTD8,
                    WTD, NJ_TD)
    bb0 = _pad_bias(np.asarray(inputs["bu_b0"], np.float32), OBU8, WBU, NJ_BU)
    bb1 = _pad_bias(np.asarray(inputs["bu_b1"], np.float32), OBU8, WBU, NJ_BU)
    bb2 = _pad_bias(np.asarray(inputs["bu_b2"], np.float32), OBU8, WBU, NJ_BU)

    # conv weights: device ci order = [h (0:32) -> torch ci 16..47, bu -> 0..15]
    # packed for 2-tap passes: wg2[n, iy] rows 0:48 = tap (dy,-1),
    # rows 48:96 = tap (dy,0); wg1[n, iy] = tap (dy,+1).
    # conv input rows: [h natural | bu l-major: device slot q holds torch
    # bu channel 2*(q%8) + q//8], matching the l-major reload layout
    pi16 = 2 * (np.arange(IND) % 8) + np.arange(IND) // 8
    ci_perm = np.concatenate([np.arange(IND, CIN), pi16])
    Wg = np.asarray(inputs["Wg"], np.float32)
    Wc = np.asarray(inputs["Wc"], np.float32)
    wg2 = np.zeros((N, 3, KPAIR, 2 * HID), NPBF)
    wg1 = np.zeros((N, 3, CIN, 2 * HID), NPBF)
    wc2 = np.zeros((N, 3, KPAIR, HID), NPBF)
    wc1 = np.zeros((N, 3, CIN, HID), NPBF)
    for iy in range(3):
        for n in range(N):
            wg2[n, iy, 0:CIN] = Wg[n][:, ci_perm, iy, 0].T
            wg2[n, iy, 64:] = Wg[n][:, ci_perm, iy, 1].T
            wg1[n, iy] = Wg[n][:, ci_perm, iy, 2].T
            wc2[n, iy, 0:CIN] = Wc[n][:, ci_perm, iy, 0].T
            wc2[n, iy, 64:] = Wc[n][:, ci_perm, iy, 1].T
            wc1[n, iy] = Wc[n][:, ci_perm, iy, 2].T

    permh = _feat_perm(HID)
    fc1 = np.asarray(inputs["fc1_w"], np.float32)     # (100, 6272)
    fc1p = np.zeros((NCORES * OFC, fc1.shape[1]), np.float32)
    fc1p[0:100] = fc1
    # [6272, 104] -> perm -> [KH, KP, 104] -> [KP, KH, 104] bf16, col-sharded
    fc1t_full = fc1p.T[permh.reshape(-1)].reshape(KH, KP, NCORES * OFC)
    fc1t_full = fc1t_full.transpose(1, 0, 2).astype(NPBF)
    fc1t = [np.ascontiguousarray(fc1t_full[:, :, c * OFC:(c + 1) * OFC])
            for c in range(NCORES)]
    fc2t = np.ascontiguousarray(np.asarray(inputs["fc2_w"], np.float32).T)  # (100, 10)

    common = {
        "xt": xt,
        "wg2": wg2, "wg1": wg1, "wc2": wc2, "wc1": wc1,
        "bg": np.asarray(inputs["bg"], np.float32),
        "bc": np.asarray(inputs["bc"], np.float32),
        "fc1b": np.asarray(inputs["fc1_b"], np.float32).reshape(100, 1),
        "fc2t": fc2t,
        "fc2b": np.asarray(inputs["fc2_b"], np.float32).reshape(10, 1),
        "ident": np.eye(32, dtype=np.float32).astype(NPBF),
    }
    in_maps = []
    for c in range(NCORES):
        m = dict(common)
        m.update({
            "tw0": tw0[c], "tw1": tw1[c], "bw0": bw0[c],
            "bw1": bw1[c], "bw2": bw2[c],
            "tb0": tb0[c], "tb1": tb1[c],
            "bb0": bb0[c], "bb1": bb1[c],
            "bb2": bb2[c], "fc1t": fc1t[c],
        })
        in_maps.append(m)
    return in_maps


def get_graph():
    if "nc" not in _CACHED:
        _CACHED["nc"] = build_graph()
    return _CACHED["nc"]


def kernel(**inputs):
    nc = get_graph()
    in_maps = prep_inputs(inputs)
    res = bass_utils.run_bass_kernel_spmd(nc, in_maps, core_ids=list(range(NCORES)))
    out_t = np.asarray(res.results[0]["out"]).reshape(10, B)
    return np.ascontiguousarray(out_t.T).astype(np.float32)



# revision 69
# speedup vs baseline: 1.4566x; 1.4566x over previous
"""Distributed Trainium2 (8 NeuronCores) kernel for the 3-node ConvGRU
message-passing network.

Strategy (memory-bound: the five big projection matrices dominate traffic):
  - td projections (the two largest matrices, 118 MB f32 each) run in
    fp8e4m3 with DoubleRow perf mode: weights stored [98, 32 k-pairs, 2,
    1184] (pair planes stride %16), maxpooled activations transposed into
    [98, ch-pair, 2, B] fp8 lhsT tiles -> 2x PE throughput and half the
    HBM traffic of bf16 (end-to-end error improves: td noise is strongly
    compressed by the gates). bu projections + everything else stay bf16
    (bu in fp8 fails the error gate); PSUM accumulates f32.
  - Weights tensor-sharded across the 8 cores by output feature, stored
    partition-major so each streaming DMA reads contiguous slabs.
  - Convs: 6 accumulating matmul passes per conv (3x K=112 pair-taps using
    an x+1-shifted copy of the input stacked on partitions 64:112, plus 3x
    K=48 single taps) instead of 9 passes of K=48.
  - Big matmuls: lhsT = transposed activations per (slab, ch) chunk
    (s-major, so the first half of each contraction only needs maxpool
    slab 0), rhs = streamed weight tiles; outputs evacuated to [B, O] sba,
    transposed in W-wide blocks, biased, and bounced to DRAM with output
    feature f living at matmul column (f%nj)*W + f//nj so the bounce DMA
    writes nj*B contiguous bytes per partition (one descriptor each).
    Two AllGathers per timestep: B(u)={td1(u),bu1(u)} -> cell1(u);
    A(u)={bu2(u),bu0(u+1),td0(u+1)} -> cell2(u), cell0(u+1). (A per-matrix
    split into 4 gathers/step measured slightly worse on HW.)
  - Latency hiding: bu0(u+1) (x-only input) fills the gather-B window;
    cell0(u+1) runs in its own comb2/rz2 buffers, deferred so it
    interleaves with td1(u+1)'s matmul stream (generator zip); maxpool
    x AND y passes are emitted inside the cell right behind each GRU
    update half (into per-node mpq buffers), so the next stream's slab-0
    transposes are never queued behind the second update half; slab-1
    transposes overlap the first half of each contraction via big_matmul's
    mid= hook; cell assembly is fused adds (h+td etc.) with the
    x+1-shifted block written directly from sources (no serial ACT shift),
    bu reloaded straight into comb[HID:CIN] (td added in place, subtracted
    back out for the cand conv), the bu-independent h-part emitted before
    the reload lands, and the first 3 conv chunks (rows <= 8) emitted
    right after the first assembly half; the first weight tile's DMA is
    split so the stream's first matmul starts sooner.
  - Queue discipline: weight streams on SP/HWDGE, bounce writes colocated
    with the collectives on the gpsimd/SWDGE queue, reloads alternate
    ACT/gpsimd queues so their DGE-config times overlap, maxpool/assembly
    on DVE. Reload channels are l-major interleaved across cores (device
    slot l*8+c = core c's l-th channel; td shards own torch h channels
    {c, 8+c, 16+c, 24+c} + bu {2c, 2c+1}), so one DMA per channel group
    covers all 8 cores: 2 DMAs per bu reload / 6 per td instead of 8.
  - fc1 is output-sharded (13 of 104 padded outputs per core, full
    contraction) with one tiny f32 partial AllGather at the end.

Measured: single-core TimelineSim 2.05 ms for the full t_end=10 run
(baseline 2.31 ms); 8-core HW rel err 8.1e-3 (gate 2e-2); HBM weight
traffic ~25.6 MB/core/step vs 39.8 baseline.

Self-contained: hardcodes all shapes; host-side numpy does the sharding,
permutation, bf16/fp8 conversion and final unshard.
"""
import sys
import numpy as np
import ml_dtypes

for _p in ("/opt/trn_rl_repo", "/opt/pypackages",
           "/root/.axon_site", "/root/.axon_site/_ro/trn_rl_repo",
           "/root/.axon_site/_ro/pypackages"):
    if _p not in sys.path:
        sys.path.append(_p)

import concourse.bass as bass
import concourse.bacc as bacc
import concourse.mybir as mybir
import concourse.tile as tile
from concourse import bass_utils

F32 = mybir.dt.float32
F32R = mybir.dt.float32r
BF16 = mybir.dt.bfloat16
F8 = mybir.dt.float8e4
AF = mybir.ActivationFunctionType
GDT = BF16                   # dtype of the gather path (bounce + reload)
NPBF = ml_dtypes.bfloat16
NP8 = ml_dtypes.float8_e4m3

NCORES = 8
B, T, C, H, W = 16, 8, 3, 14, 14
HID, IND, N = 32, 16, 3
CIN = IND + HID              # 48 conv input channels
YP = XP = 16                 # padded spatial
# conv valid output flat window (phys coords, (y*XP+x)*B): (1,1)..(14,14)
WSTART = (1 * XP + 1) * B
WLEN = ((14 * XP + 14) - (1 * XP + 1) + 1) * B    # 3552
FLAT = YP * XP * B           # 4096

KP = 98                      # partitions per feature chunk (7 y-rows x 14 x)
KH = 2 * HID                 # 64 chunks for hidden-sized contraction (6272)
KHP = KH // 2                # 32 physical chunks for fp8 DoubleRow (k-pairs)
KX = 2 * C                   # 6 chunks for x contraction (588)
O_TD = (IND + HID) * H * W   # 9408
O_BU = IND * H * W           # 3136
OTD8 = O_TD // NCORES        # 1176 = 6 channels
OBU8 = O_BU // NCORES        # 392  = 2 channels
NJ_TD = 10                   # o-blocks per td shard
NJ_BU = 4
WTD = 118                    # o-block width (transpose partitions); feature
WBU = 98                     # f sits at block j=f%nj, row p=f//nj, so the
OTD8P = WTD * NJ_TD          # bounce DMA writes nj*B contiguous bytes per
                             # partition (1180: 4 zero-pad cols for td)
OQ8 = 1184                   # fp8 DoubleRow pair-plane stride (%16 == 0)
GRP_TD = 8                   # weight K-chunks per DMA (td)
GRP_BU = 8
OFC = 13                     # fc1 output columns per core (8*13=104 >= 100)
KPAIR = 112                  # pair-tap conv K: 48 + 16 zero pad + 48 shifted

_CACHED = {}


# ---------------------------------------------------------------- graph ----
def build_graph(t_end=T + N - 1, debug_h=False, no_cc=False, split_cc=False):
    # split_cc: False (default) = one gather per round — measured best.
    # "A" = also gather bu2/bu0 separately under td0's stream (+225us on
    # HW); True = split both rounds (+125us). Extra collective launches
    # cost more than the overlap they buy on this hardware.
    sa = split_cc in (True, "A")
    sb = split_cc is True
    nc = bacc.Bacc(None, target_bir_lowering=False, debug=False,
                   num_devices=NCORES)

    dp = nc.declare_dram_parameter
    # streamed weight shards, partition-major [98, K, O/8] bf16
    # td weights fp8e4m3 DoubleRow-packed: [98, 32 k-pairs, 2, OQ8] — the
    # two logical k rows of a pair are separate planes (BIR wants the
    # rhs AP's second dim Num=2 with plane stride % 16 == 0)
    tw0 = dp("tw0", [KP, KHP, 2, OQ8], F8, isOutput=False)
    tw1 = dp("tw1", [KP, KHP, 2, OQ8], F8, isOutput=False)
    bw0 = dp("bw0", [KP, KX, OBU8], BF16, isOutput=False)
    bw1 = dp("bw1", [KP, KH, OBU8], BF16, isOutput=False)
    bw2 = dp("bw2", [KP, KH, OBU8], BF16, isOutput=False)
    # bias shards (o-chunk padded) f32
    tb0 = dp("tb0", [NJ_TD, 128], F32, isOutput=False)
    tb1 = dp("tb1", [NJ_TD, 128], F32, isOutput=False)
    bb0 = dp("bb0", [NJ_BU, 128], F32, isOutput=False)
    bb1 = dp("bb1", [NJ_BU, 128], F32, isOutput=False)
    bb2 = dp("bb2", [NJ_BU, 128], F32, isOutput=False)
    # pre-transposed input x: [t, 98, k, B] bf16 (partition-major)
    xt_in = dp("xt", [T, KP, KX, B], BF16, isOutput=False)
    # conv weights: pair-tap packed [node, dy, 96, co] + single-tap [.., 48, co]
    wg2_in = dp("wg2", [N, 3, KPAIR, 2 * HID], BF16, isOutput=False)
    wg1_in = dp("wg1", [N, 3, CIN, 2 * HID], BF16, isOutput=False)
    wc2_in = dp("wc2", [N, 3, KPAIR, HID], BF16, isOutput=False)
    wc1_in = dp("wc1", [N, 3, CIN, HID], BF16, isOutput=False)
    bg_in = dp("bg", [N, 2 * HID], F32, isOutput=False)
    bc_in = dp("bc", [N, HID], F32, isOutput=False)
    # fc (fc1 output-sharded: this core's OFC output columns)
    fc1_in = dp("fc1t", [KP, KH, OFC], BF16, isOutput=False)
    fc1b_in = dp("fc1b", [100, 1], F32, isOutput=False)
    fc2_in = dp("fc2t", [100, 10], F32, isOutput=False)
    fc2b_in = dp("fc2b", [10, 1], F32, isOutput=False)
    ident_in = dp("ident", [32, 32], BF16, isOutput=False)
    out_ext = dp("out", [10, B], F32, isOutput=True)
    dbg_ext = dp("dbg", [N, HID, 14, 14, B], F32, isOutput=True) if debug_h else None

    from contextlib import ExitStack
    with tile.TileContext(nc) as tc, ExitStack() as ctx:
        consts = ctx.enter_context(tc.tile_pool(name="consts", bufs=1))
        wtd_pool = ctx.enter_context(tc.tile_pool(name="wtd", bufs=2))
        wbu_pool = ctx.enter_context(tc.tile_pool(name="wbu", bufs=2))
        mpt_pool = ctx.enter_context(tc.tile_pool(name="mpt", bufs=4))
        pst_pool = ctx.enter_context(tc.tile_pool(name="pst", bufs=2, space="PSUM"))
        acc_pool = ctx.enter_context(tc.tile_pool(name="accp", bufs=1, space="PSUM"))
        conv_pool = ctx.enter_context(tc.tile_pool(name="convp", bufs=2, space="PSUM"))
        sbacc_pool = ctx.enter_context(tc.tile_pool(name="sbacc", bufs=1))
        outt_pool = ctx.enter_context(tc.tile_pool(name="outt", bufs=2))
        dram = ctx.enter_context(tc.tile_pool(name="dram", bufs=1, space="DRAM"))

        # ---------------- constants ----------------
        ident = consts.tile([32, 32], BF16)
        nc.sync.dma_start(ident[:], ident_in[:])
        wg2_sb = consts.tile([KPAIR, N, 3, 2 * HID], BF16)
        nc.sync.dma_start(wg2_sb[:], wg2_in[:].rearrange("n s c o -> c n s o"))
        wg1_sb = consts.tile([CIN, N, 3, 2 * HID], BF16)
        nc.sync.dma_start(wg1_sb[:], wg1_in[:].rearrange("n s c o -> c n s o"))
        wc2_sb = consts.tile([KPAIR, N, 3, HID], BF16)
        nc.sync.dma_start(wc2_sb[:], wc2_in[:].rearrange("n s c o -> c n s o"))
        wc1_sb = consts.tile([CIN, N, 3, HID], BF16)
        nc.sync.dma_start(wc1_sb[:], wc1_in[:].rearrange("n s c o -> c n s o"))
        bg_sb = consts.tile([2 * HID, N], F32)
        nc.sync.dma_start(bg_sb[:], bg_in[:].rearrange("n o -> o n"))
        bc_sb = consts.tile([HID, N], F32)
        nc.sync.dma_start(bc_sb[:], bc_in[:].rearrange("n o -> o n"))
        tb0_sb = consts.tile([128, NJ_TD], F32)
        nc.sync.dma_start(tb0_sb[:], tb0[:].rearrange("j p -> p j"))
        tb1_sb = consts.tile([128, NJ_TD], F32)
        nc.sync.dma_start(tb1_sb[:], tb1[:].rearrange("j p -> p j"))
        bb0_sb = consts.tile([128, NJ_BU], F32)
        nc.sync.dma_start(bb0_sb[:], bb0[:].rearrange("j p -> p j"))
        bb1_sb = consts.tile([128, NJ_BU], F32)
        nc.sync.dma_start(bb1_sb[:], bb1[:].rearrange("j p -> p j"))
        bb2_sb = consts.tile([128, NJ_BU], F32)
        nc.sync.dma_start(bb2_sb[:], bb2[:].rearrange("j p -> p j"))
        fc2_sb = consts.tile([100, 10], F32)
        nc.sync.dma_start(fc2_sb[:], fc2_in[:])
        fc1b_sb = consts.tile([100, 1], F32)
        nc.sync.dma_start(fc1b_sb[:], fc1b_in[:])
        fc2b_sb = consts.tile([10, 1], F32)
        nc.sync.dma_start(fc2b_sb[:], fc2b_in[:])
        xt_all = consts.tile([KP, T, KX, B], BF16)
        nc.sync.dma_start(xt_all[:], xt_in[:].rearrange("t p k b -> p t k b"))

        # ------------- dedicated activation tensors (shared/aliased) -------
        h = [consts.tile([HID, YP, XP, B], BF16, name=f"h{i}", tag=f"h{i}")
             for i in range(N)]
        # conv input, pair-tap stacked: [0:48]=comb, [64:112]=comb shifted
        # +1 x; [48:64] stays zero (partition bases must be 0/32/64/96).
        # comb2/rz2 let cell0 interleave with cell2 inside round A.
        comb = consts.tile([KPAIR, YP, XP, B], BF16)
        comb2 = consts.tile([KPAIR, YP, XP, B], BF16)
        rz = consts.tile([2 * HID, YP, XP, B], BF16)  # gates; [0:HID] doubles
        #   as cand / maxpool output / relu buffer
        rz2 = consts.tile([2 * HID, YP, XP, B], BF16)
        td_buf = [consts.tile([CIN, YP, XP, B], GDT, name=f"td{i}", tag=f"td{i}")
                  for i in range(2)]
        for tt in h + td_buf + [rz, rz2, comb, comb2]:
            nc.vector.memset(tt[:], 0.0)

        # maxpool scratch: per-cell x-pass scratch (tmq2 isolates cell2,
        # which interleaves with cell0) and per-node maxpool output, so the
        # y-passes can be emitted inside the cell right after each update
        # half without clobbering a not-yet-transposed earlier maxpool.
        tmq = consts.tile([HID, YP, XP, B], BF16, name="tmq", tag="tmq")
        tmq2 = consts.tile([HID, YP, XP, B], BF16, name="tmq2", tag="tmq2")
        mpq = [consts.tile([HID, YP, XP, B], BF16, name=f"mpq{i}",
                           tag=f"mpq{i}") for i in range(N)]

        # ---------------- helpers ----------------
        def mp_x_pass(src, tq=None, y0=1, y1=15, eng=None):
            tq = tmq if tq is None else tq
            eng = nc.vector if eng is None else eng
            eng.tensor_max(tq[0:HID, y0:y1, 2:14, :], src[0:HID, y0:y1, 1:13, :], src[0:HID, y0:y1, 2:14, :])
            eng.tensor_max(tq[0:HID, y0:y1, 2:14, :], tq[0:HID, y0:y1, 2:14, :], src[0:HID, y0:y1, 3:15, :])
            eng.tensor_max(tq[0:HID, y0:y1, 1:2, :], src[0:HID, y0:y1, 1:2, :], src[0:HID, y0:y1, 2:3, :])
            eng.tensor_max(tq[0:HID, y0:y1, 14:15, :], src[0:HID, y0:y1, 13:14, :], src[0:HID, y0:y1, 14:15, :])

        def mp_y_pass(s, tq, mp):
            if s == 0:
                nc.vector.tensor_max(mp[0:HID, 2:8, 1:15, :], tq[0:HID, 1:7, 1:15, :], tq[0:HID, 2:8, 1:15, :])
                nc.vector.tensor_max(mp[0:HID, 2:8, 1:15, :], mp[0:HID, 2:8, 1:15, :], tq[0:HID, 3:9, 1:15, :])
                nc.vector.tensor_max(mp[0:HID, 1:2, 1:15, :], tq[0:HID, 1:2, 1:15, :], tq[0:HID, 2:3, 1:15, :])
            else:
                nc.vector.tensor_max(mp[0:HID, 8:14, 1:15, :], tq[0:HID, 7:13, 1:15, :], tq[0:HID, 8:14, 1:15, :])
                nc.vector.tensor_max(mp[0:HID, 8:14, 1:15, :], mp[0:HID, 8:14, 1:15, :], tq[0:HID, 9:15, 1:15, :])
                nc.vector.tensor_max(mp[0:HID, 14:15, 1:15, :], tq[0:HID, 13:14, 1:15, :], tq[0:HID, 14:15, 1:15, :])

        def transpose_slab(src, s, mt, mt8=None):
            """slab s of src[0:HID] (y rows 1+7s .. 8+7s) -> mt [98, HID, B]
            (bf16) and/or mt8 [98, HID/2, 2, B] (fp8 pair planes for
            DoubleRow lhsT). PE transpose needs a single-free-dim input, so
            first repack the (y, x)-strided valid slice contiguously."""
            y0 = 1 + 7 * s
            stg = mpt_pool.tile([HID, B, KP], BF16, tag="stg", name="stg", bufs=2)
            nc.vector.tensor_copy(
                stg[:].rearrange("c b (y x) -> c y x b", y=7, x=14),
                src[0:HID, y0:y0 + 7, 1:15, :])
            for b in range(B):
                pt = pst_pool.tile([128, HID], BF16, tag="psT", name="ptt")
                nc.tensor.transpose(pt[:KP, 0:HID], stg[:, b, :].opt(),
                                    ident[0:HID, 0:HID])
                if mt is not None:
                    nc.scalar.activation(mt[:, 0:HID, b], pt[:KP, 0:HID],
                                         AF.Copy)
                if mt8 is not None:
                    nc.scalar.activation(
                        mt8[:, :, :, b],
                        pt[:KP, 0:HID].rearrange("p (cc i) -> p cc i", i=2),
                        AF.Copy)

        def maxpool_transpose(node, f16=True, f8=False):
            """mpq[node] (maxpool of h[node]; x/y passes were emitted inside
            the cell, right behind each update half) -> (m16, m8,
            finish_slab1). Slab 0 transposes emit immediately; the returned
            callback emits slab 1 and is passed to big_matmul's mid= hook so
            the first (s-major) half of the contraction overlaps it."""
            out = ([mpt_pool.tile([KP, HID, B], BF16, tag="mpt",
                                  name=f"mpt{s}") for s in range(2)]
                   if f16 else None)
            out8 = ([mpt_pool.tile([KP, HID // 2, 2, B], F8, tag="mpt8",
                                   name=f"mpt8{s}") for s in range(2)]
                    if f8 else None)
            transpose_slab(mpq[node], 0, out and out[0], out8 and out8[0])

            def finish():
                transpose_slab(mpq[node], 1, out and out[1],
                               out8 and out8[1])
            return out, out8, finish

        def transpose_feat(src):
            out = [mpt_pool.tile([KP, HID, B], BF16, tag="mpt", name=f"mpt{s}")
                   for s in range(2)]
            for s in range(2):
                transpose_slab(src, s, out[s])
            return out

        def big_matmul_gen(nk, o8, W_, nj, lhsT_of, w_dram, grp, bias_sb,
                           agin, row_off, mid=None, dr=False):
            """Streamed o-sharded matmul: out.T[o8, B] = W_shard @ act (+bias),
            written into agin[row_off : row_off+o8, :]. Device feature order:
            feature f lives at matmul column (f%nj)*W_ + f//nj, so agin row f
            sits at (partition f//nj, j=f%nj) of outT and the bounce DMA is
            one nj*B-byte descriptor per partition. k order is s-major
            (chunks 0..nk/2-1 use activation slab 0)."""
            nslice = (o8 + 511) // 512
            pacc = acc_pool.tile([B, 512 * nslice], F32,
                                 tag=("acc" if nslice > 1 else "accbu"),
                                 name="pacc")
            for g in range(0, nk, grp):
                if mid is not None and g == nk // 2:
                    mid()
                pool = wtd_pool if o8 == OTD8P else wbu_pool
                if dr:
                    wt = pool.tile([KP, grp, 2, OQ8], F8, tag="w", name="wt")
                    wsrc = w_dram[:, g:g + grp, :, :]
                else:
                    wt = pool.tile([KP, grp, o8], BF16, tag="w", name="wt")
                    wsrc = w_dram[:, g:g + grp, :]
                if g == 0 and grp >= 4:
                    # split the first tile's DMA so matmul 0 starts sooner
                    h1 = grp // 4
                    nc.sync.dma_start(wt[:, 0:h1], wsrc[:, 0:h1])
                    nc.sync.dma_start(wt[:, h1:grp], wsrc[:, h1:grp])
                else:
                    nc.sync.dma_start(wt[:], wsrc)
                for j in range(grp):
                    k = g + j
                    for sl in range(nslice):
                        o0 = sl * 512
                        ln = min(512, o8 - o0)
                        if dr:
                            nc.tensor.matmul(
                                pacc[:, o0:o0 + ln],
                                lhsT_of(k),
                                wt[:, j, :, o0:o0 + ln],
                                start=(k == 0), stop=(k == nk - 1),
                                perf_mode=mybir.MatmulPerfMode.DoubleRow,
                            )
                        else:
                            nc.tensor.matmul(
                                pacc[:, o0:o0 + ln],
                                lhsT_of(k).opt(),
                                wt[:, j, o0:o0 + ln].opt(),
                                start=(k == 0), stop=(k == nk - 1),
                            )
                yield
            sba = sbacc_pool.tile([B, o8], BF16, tag="sba", name="sba", bufs=2)
            for sl in range(nslice):
                o0 = sl * 512
                ln = min(512, o8 - o0)
                nc.scalar.activation(sba[:, o0:o0 + ln], pacc[:, o0:o0 + ln],
                                     AF.Copy)
            outT = outt_pool.tile([128, nj, B], GDT, tag="outT", name="outT")
            for jj in range(nj):
                pt = pst_pool.tile([128, HID], BF16, tag="psT", name="pt2")
                nc.tensor.transpose(pt[:W_, 0:B], sba[:, jj * W_: (jj + 1) * W_],
                                    ident[0:B, 0:B])
                nc.scalar.activation(outT[:W_, jj, :], pt[:W_, 0:B], AF.Identity,
                                     bias=bias_sb[0:W_, jj:jj + 1])
            nc.gpsimd.dma_start(
                agin[row_off: row_off + o8, :].rearrange(
                    "(p j) b -> p j b", j=nj),
                outT[0:W_, :, :])

        def big_matmul(*a, **kw):
            for _ in big_matmul_gen(*a, **kw):
                pass

        def do_gather(agin, agout):
            if no_cc:
                # sim-only stand-in for the AllGather: flat views (one
                # descriptor per copy) + log2 doubling
                af = agin[:].rearrange("r b -> (r b)")
                of = agout[:].rearrange("c r b -> c (r b)")
                nc.gpsimd.dma_start(of[0], af)
                nc.gpsimd.dma_start(of[1], of[0])
                nc.gpsimd.dma_start(of[2:4], of[0:2])
                nc.gpsimd.dma_start(of[4:8], of[0:4])
            else:
                nc.gpsimd.collective_compute(
                    "AllGather", mybir.AluOpType.bypass,
                    replica_groups=[list(range(NCORES))],
                    ins=[agin.opt()], outs=[agout.opt()])

        def reload(buf, agout, row_off, nch_l, pbase=0):
            """agout [8, rows, B] o-major (bf16) -> buf partitions
            [pbase : pbase+8*nch_l], interior 14x14. Channels are l-major
            interleaved (device slot l*8+c holds core c's l-th channel), so
            ONE DMA per channel group covers all 8 cores: dst partitions
            [l*8, (l+1)*8) are contiguous and the core stride in agout is
            uniform. DMAs alternate ACT/gpsimd queues."""
            for l in range(nch_l):
                srcv = agout[:, row_off + l * 196: row_off + (l + 1) * 196,
                             :].rearrange("c (y x) b -> c y x b", y=14, x=14)
                eng = nc.scalar if l % 2 == 0 else nc.gpsimd
                eng.dma_start(
                    buf[pbase + 8 * l: pbase + 8 * (l + 1), 1:15, 1:15, :],
                    srcv)

        def conv6_gen(w2_of, w1_of, nco, bias_ap, out_t, act_fn, cmb):
            inp_f = cmb[:].rearrange("c y x b -> c (y x b)")
            out_f = out_t.rearrange("c y x b -> c (y x b)")
            q = 0
            while q < WLEN:
                ln = min(512, WLEN - q)
                pc = conv_pool.tile([nco, 512], F32, tag="conv", name="pc")
                for i, dy in enumerate((-1, 0, 1)):
                    offp = (dy * XP - 1) * B     # pair taps (dy,-1)+(dy,0)
                    nc.tensor.matmul(
                        pc[:, 0:ln],
                        w2_of(i).opt(),
                        inp_f[0:KPAIR, WSTART + q + offp: WSTART + q + offp + ln],
                        start=(i == 0), stop=False,
                    )
                    offs = (dy * XP + 1) * B     # single tap (dy,+1)
                    nc.tensor.matmul(
                        pc[:, 0:ln],
                        w1_of(i).opt(),
                        inp_f[0:CIN, WSTART + q + offs: WSTART + q + offs + ln],
                        start=False, stop=(i == 2),
                    )
                nc.scalar.activation(out_f[:, WSTART + q: WSTART + q + ln],
                                     pc[:, 0:ln], act_fn, bias=bias_ap)
                q += ln
                yield

        def asm_gate_h(node, td_t, y0, y1, cmb):
            """h-only part of the gate-conv assembly (independent of the
            gathered bu, so it can be emitted before the bu reload lands):
            cmb[0:HID]=h(+td) plus the x+1-shifted copy at 64:96."""
            hh = h[node]
            cf = cmb[:].rearrange("c y x b -> c (y x b)")
            hf = hh[:].rearrange("c y x b -> c (y x b)")
            f0 = max(0, y0 * XP * B - B)
            f1 = min(FLAT - B, y1 * XP * B - B)
            if td_t is not None:
                tf = td_t[:].rearrange("c y x b -> c (y x b)")
                nc.vector.tensor_add(cmb[0:HID, y0:y1, :, :],
                                     hh[:, y0:y1, :, :],
                                     td_t[0:HID, y0:y1, :, :])
                nc.vector.tensor_add(cf[64:64 + HID, f0:f1],
                                     hf[0:HID, f0 + B:f1 + B],
                                     tf[0:HID, f0 + B:f1 + B])
            else:
                nc.vector.tensor_copy(cmb[0:HID, y0:y1, :, :],
                                      hh[:, y0:y1, :, :])
                nc.vector.tensor_copy(cf[64:64 + HID, f0:f1],
                                      hf[0:HID, f0 + B:f1 + B])

        def asm_gate_bu(node, td_t, y0, y1, cmb):
            """bu part: cmb[HID:CIN]=bu(+td) (bu lands there via reload)
            plus the shifted copy at 96:112."""
            cf = cmb[:].rearrange("c y x b -> c (y x b)")
            f0 = max(0, y0 * XP * B - B)
            f1 = min(FLAT - B, y1 * XP * B - B)
            if td_t is not None:
                nc.vector.tensor_add(cmb[HID:CIN, y0:y1, :, :],
                                     cmb[HID:CIN, y0:y1, :, :],
                                     td_t[HID:CIN, y0:y1, :, :])
            nc.vector.tensor_copy(cf[64 + HID:KPAIR, f0:f1],
                                  cf[HID:CIN, f0 + B:f1 + B])

        def asm_cand(node, td_t, y0, y1, cmb, rzt):
            """cmb -> cand-conv input rows [y0,y1): [r*h | bu] + shifts.
            cmb[HID:CIN] holds bu+td; subtract td back out to recover bu."""
            hh = h[node]
            cf = cmb[:].rearrange("c y x b -> c (y x b)")
            rf = rzt[:].rearrange("c y x b -> c (y x b)")
            hf = hh[:].rearrange("c y x b -> c (y x b)")
            f0 = max(0, y0 * XP * B - B)
            f1 = min(FLAT - B, y1 * XP * B - B)
            nc.vector.tensor_mul(cmb[0:HID, y0:y1, :, :],
                                 rzt[0:HID, y0:y1, :, :], hh[:, y0:y1, :, :])
            nc.vector.tensor_mul(cf[64:64 + HID, f0:f1],
                                 rf[0:HID, f0 + B:f1 + B],
                                 hf[0:HID, f0 + B:f1 + B])
            if td_t is not None:
                nc.vector.tensor_sub(cmb[HID:CIN, y0:y1, :, :],
                                     cmb[HID:CIN, y0:y1, :, :],
                                     td_t[HID:CIN, y0:y1, :, :])
                nc.vector.tensor_copy(cf[64 + HID:KPAIR, f0:f1],
                                      cf[HID:CIN, f0 + B:f1 + B])

        def cell_gen(node, td_t, xq, cmb, rzt):
            """GRU cell update of h[node]; bu arrives in cmb[HID:CIN] via
            reload. The first 3 conv chunks (rows <= 8) are emitted right
            after the first assembly half so PE starts while the second half
            assembles; generator yields let an interleaved partner (another
            cell or a matmul stream) fill the gaps. The maxpool x AND y
            passes of the fresh h are emitted right behind each update half
            into mpq[node], so the next round's slab-0 transposes aren't
            queued behind the second update half."""
            hh = h[node]
            asm_gate_h(node, td_t, 0, 9, cmb)
            asm_gate_h(node, td_t, 9, 16, cmb)
            asm_gate_bu(node, td_t, 0, 9, cmb)
            yield
            gg = conv6_gen(lambda i: wg2_sb[:, node, i, :],
                           lambda i: wg1_sb[:, node, i, :],
                           2 * HID, bg_sb[:, node:node + 1], rzt[:],
                           AF.Sigmoid, cmb)
            for _ in range(3):
                next(gg)
                yield
            asm_gate_bu(node, td_t, 9, 16, cmb)
            yield
            yield from gg
            asm_cand(node, td_t, 0, 9, cmb, rzt)
            yield
            cg = conv6_gen(lambda i: wc2_sb[:, node, i, :],
                           lambda i: wc1_sb[:, node, i, :],
                           HID, bc_sb[:, node:node + 1],
                           rzt[0:HID, :, :, :], AF.Tanh, cmb)
            for _ in range(3):
                next(cg)
                yield
            asm_cand(node, td_t, 9, 16, cmb, rzt)
            yield
            yield from cg
            for s, (y0, y1) in enumerate(((1, 9), (9, 15))):
                hv = hh[:, y0:y1, 1:15, :]
                cv = rzt[0:HID, y0:y1, 1:15, :]
                # z lives at base partition 32; DVE tensor-tensor ops need
                # equal base partitions, so stage it at base 0 in cmb.
                zc = cmb[0:HID, y0:y1, 1:15, :]
                nc.vector.tensor_copy(zc, rzt[HID:2 * HID, y0:y1, 1:15, :])
                nc.vector.tensor_sub(cv, cv, hv)
                nc.vector.tensor_mul(cv, cv, zc)
                nc.vector.tensor_add(hv, hv, cv)
                mp_x_pass(hh, xq, y0, y1)
                mp_y_pass(s, xq, mpq[node])
                yield

        def cell(node, td_t, xq, cmb, rzt):
            for _ in cell_gen(node, td_t, xq, cmb, rzt):
                pass

        def zip2(ga, gb, ratio=2):
            """Interleave two generators, giving `ga` (the critical-path
            cell) `ratio` steps per `gb` step."""
            while ga is not None or gb is not None:
                for _ in range(ratio):
                    if ga is not None:
                        try:
                            next(ga)
                        except StopIteration:
                            ga = None
                if gb is not None:
                    try:
                        next(gb)
                    except StopIteration:
                        gb = None

        # ------------- round schedule: 2 collectives per timestep -------------
        # Round u (u = timestep of cell1/cell2):
        #   B(u): gather {td1(u) [u>=2], bu1(u)} -> cell1(u)
        #   A(u): gather {bu2(u) [u>=1], bu0(u+1) [u+1<T], td0(u+1) [2<=u+1<T]}
        #         -> cell2(u) [u>=1], cell0(u+1) [u+1<T]
        # bu2(u) and td0(u+1) share mp(h1@u) (one maxpool+transpose).
        # cell0(u+1) is returned as a pending generator and interleaved with
        # td1(u+1)'s matmul stream at the start of the next round_B.
        def lam(m):
            return lambda k, mm=m: mm[k // HID][:, (k % HID), :]

        def lam8(m8):
            # physical chunk kk -> [98, 2, B] fp8 lhsT: channel pair
            # (2cc, 2cc+1) of slab kk//16 as two B-column planes
            return lambda kk, mm=m8: mm[kk // (HID // 2)][
                :, kk % (HID // 2), :, :]

        def mk_ag(rows, name):
            agin = dram.tile([rows, B], GDT, name=f"agin{name}",
                             tag=f"agin{name}")
            agout = dram.tile([NCORES, rows, B], GDT, name=f"agout{name}",
                              tag=f"agout{name}",
                              addr_space="Local" if no_cc else "Shared")
            return agin, agout

        # Per-matrix gathers: the first collective of each round launches as
        # soon as its matrix's stream ends and completes under the next
        # stream (round B: td1-gather under bu1's stream; round A: bu2/bu0-
        # gather under td0's stream, and td0's own gather under cell2's
        # convs). Only bu1's small gather is exposed per step.
        def round_A_pre(u):
            hbu2 = 1 <= u < t_end
            hbu0 = u + 1 < min(T, t_end)
            htd0 = 2 <= u + 1 < min(T, t_end)
            if not (hbu2 or hbu0 or htd0):
                return None
            st = dict(hbu2=hbu2, hbu0=hbu0, htd0=htd0,
                      ro_bu0=(OBU8 if hbu2 else 0))
            rows1 = st["ro_bu0"] + (OBU8 if hbu0 else 0)
            st["rows1"] = rows1
            if sa:
                if rows1:
                    st["ag1"] = mk_ag(rows1, f"A1_{u}")
                if htd0:
                    st["ag2"] = mk_ag(OTD8P, f"A2_{u}")
            else:
                # one gather for the whole round: td0 rows follow bu2/bu0
                ag = mk_ag(rows1 + (OTD8P if htd0 else 0), f"A1_{u}")
                st["ag1"] = ag
                st["ag2"] = (ag[0], ag[1], rows1) if htd0 else None
            if hbu0:
                # bu0 depends only on x: its matmuls slot into the PE-idle
                # window while round_B's bu1 gather + reload are in flight.
                big_matmul(KX, OBU8, WBU, NJ_BU,
                           lambda k: xt_all[:, u + 1, k, :], bw0, KX,
                           bb0_sb, st["ag1"][0], st["ro_bu0"])
            return st

        def round_A_rest(u, st):
            if st is None:
                return None
            hbu2, hbu0, htd0 = st["hbu2"], st["hbu0"], st["htd0"]
            mid1 = None
            if hbu2 or htd0:
                m1, m1_8, mid1 = maxpool_transpose(1, f16=hbu2, f8=htd0)
            if hbu2:
                big_matmul(KH, OBU8, WBU, NJ_BU, lam(m1), bw2, GRP_BU,
                           bb2_sb, st["ag1"][0], 0, mid=mid1)
                mid1 = None
            if sa and st["rows1"]:
                do_gather(*st["ag1"])
            if htd0:
                ro_td = st["ag2"][2] if not sa else 0
                big_matmul(KHP, OTD8P, WTD, NJ_TD, lam8(m1_8), tw0, GRP_TD,
                           tb0_sb, st["ag2"][0], ro_td, mid=mid1, dr=True)
                mid1 = None
            if mid1 is not None:
                mid1()
            if not sa:
                do_gather(st["ag1"][0], st["ag1"][1])
            if hbu2:
                reload(comb, st["ag1"][1], 0, IND // NCORES, pbase=HID)
            if hbu0:
                reload(comb2, st["ag1"][1], st["ro_bu0"], IND // NCORES,
                       pbase=HID)
            if hbu2:
                cell(2, None, tmq2, comb, rz)
            if htd0:
                if sa:
                    do_gather(*st["ag2"])
                reload(td_buf[0], st["ag2"][1], st["ag2"][2]
                       if not sa else 0, CIN // NCORES)
            # cell0(u+1) runs in comb2/rz2, deferred so its elementwise work
            # interleaves with round_B(u+1)'s td1 matmul stream.
            if hbu0:
                return cell_gen(0, td_buf[0] if htd0 else None, tmq, comb2,
                                rz2)
            return None

        def round_B(u, pend_cell0):
            htd1 = u >= 2
            if sb:
                ag1 = mk_ag(OTD8P, f"B1_{u}") if htd1 else None
                ag2 = mk_ag(OBU8, f"B2_{u}")
                ro_bu1 = 0
            else:
                ag2 = mk_ag((OTD8P if htd1 else 0) + OBU8, f"B1_{u}")
                ag1 = ag2 if htd1 else None
                ro_bu1 = OTD8P if htd1 else 0
            if htd1:
                _, m2_8, mid2 = maxpool_transpose(2, f16=False, f8=True)
                td1_gen = big_matmul_gen(KHP, OTD8P, WTD, NJ_TD, lam8(m2_8),
                                         tw1, GRP_TD, tb1_sb, ag1[0], 0,
                                         mid=mid2, dr=True)
            else:
                td1_gen = None
            zip2(pend_cell0, td1_gen)
            if sb and htd1:
                do_gather(*ag1)
                reload(td_buf[1], ag1[1], 0, CIN // NCORES)
            m0, _, mid0 = maxpool_transpose(0)
            big_matmul(KH, OBU8, WBU, NJ_BU, lam(m0), bw1, GRP_BU,
                       bb1_sb, ag2[0], ro_bu1, mid=mid0)
            do_gather(ag2[0], ag2[1])
            stA = round_A_pre(u)   # bu0(u+1) fills the gather window
            if not sb and htd1:
                reload(td_buf[1], ag1[1], 0, CIN // NCORES)
            reload(comb, ag2[1], ro_bu1, IND // NCORES, pbase=HID)
            cell(1, td_buf[1] if htd1 else None, tmq, comb, rz)
            return stA

        pend = round_A_rest(-1, round_A_pre(-1))  # bootstrap: bu0(0)->cell0(0)
        for u in range(t_end):
            if u >= 1:
                stA = round_B(u, pend)
            else:
                zip2(pend, None)
                stA = round_A_pre(u)
            pend = round_A_rest(u, stA)
        zip2(pend, None)

        if debug_h:
            for i in range(N):
                nc.gpsimd.dma_start(dbg_ext[i], h[i][:, 1:15, 1:15, :])
        # -------- final FC head (fc1 output-sharded + partial gather) --------
        # Each core computes its OFC of the 104 (padded) fc1 outputs with the
        # full contraction, then one tiny AllGather assembles p1.
        nc.scalar.activation(rz[0:HID, :, :, :], h[2][:], AF.Relu)
        pT = transpose_feat(rz)
        pfc = acc_pool.tile([OFC, 16], F32, tag="acc", name="pfc")
        for g in range(0, KH, 8):
            wf = wtd_pool.tile([KP, 8, OFC], BF16, tag="w", name="wf")
            nc.sync.dma_start(wf[:], fc1_in[:, g:g + 8, :])
            for j in range(8):
                k = g + j
                nc.tensor.matmul(pfc[:], wf[:, j, :].opt(),
                                 pT[k // HID][:, (k % HID), :].opt(),
                                 start=(k == 0), stop=(k == KH - 1))
        pfs = sbacc_pool.tile([OFC, 16], F32, tag="pfs", name="pfs")
        nc.scalar.activation(pfs[:], pfc[:], AF.Copy)
        aginF = dram.tile([OFC, 16], F32, name="aginF", tag="aginF")
        nc.gpsimd.dma_start(aginF[:], pfs[:])
        agoutF = dram.tile([NCORES, OFC, 16], F32, name="agoutF", tag="agoutF",
                           addr_space="Local" if no_cc else "Shared")
        do_gather(aginF, agoutF)
        p1r = sbacc_pool.tile([NCORES * OFC, 16], F32, tag="p1r", name="p1r")
        nc.gpsimd.dma_start(p1r[:], agoutF[:].rearrange("c o b -> (c o) b"))
        p1 = sbacc_pool.tile([100, 16], F32, tag="p1", name="p1")
        nc.scalar.activation(p1[:], p1r[0:100, :], AF.Relu,
                             bias=fc1b_sb[:])
        pf2 = acc_pool.tile([128, HID], F32, tag="acc", name="pf2")
        nc.tensor.matmul(pf2[0:10, 0:16], fc2_sb[:], p1[:],
                         start=True, stop=True)
        osb = sbacc_pool.tile([10, 16], F32, tag="osb", name="osb")
        nc.scalar.activation(osb[:], pf2[0:10, 0:16], AF.Identity,
                             bias=fc2b_sb[:])
        nc.gpsimd.dma_start(out_ext[:], osb[:])

    nc.finalize()
    return nc


# ---------------------------------------------------------------- host ----
def _feat_perm(nch):
    """Device feature order (ch, s, p) -> torch flat feature index."""
    perm = np.zeros((nch * 2, KP), np.int64)
    for ch in range(nch):
        for s in range(2):
            k = s * nch + ch
            p = np.arange(KP)
            y = s * 7 + p // 14
            x = p % 14
            perm[k] = ch * 196 + y * 14 + x
    return perm


def _shard_w(wmat, nch_in, o8, W_, nj):
    """wmat (O, K) torch-order -> per-core [98, nk, W_*nj] bf16 shards.
    Device column j*W_ + p holds feature f = p*nj + j (zero-padded), so on
    device the bounce DMA writes agin rows partition-major."""
    perm = _feat_perm(nch_in)
    wt = wmat.T[perm.reshape(-1)].reshape(perm.shape[0], KP,
                                          wmat.shape[0]).astype(NPBF)
    o8p = W_ * nj
    c_idx = np.arange(o8p)
    f = (c_idx % W_) * nj + c_idx // W_
    valid = f < o8
    out = []
    for c in range(NCORES):
        blk = wt[:, :, c * o8:(c + 1) * o8]
        padded = np.zeros((wt.shape[0], KP, o8p), NPBF)
        padded[:, :, valid] = blk[:, :, f[valid]]
        out.append(np.ascontiguousarray(padded.transpose(1, 0, 2)))
    return out


def _shard_w8(wmat, o8, W_, nj):
    """td wmat (O, 6272) torch-order -> per-core [98, 32, 2*W_*nj] fp8e4m3
    DoubleRow shards: physical chunk kk = s*16+cc holds logical k-chunks
    (s*32+2cc, s*32+2cc+1) with their values interleaved along the output
    dim; output features permuted/padded as in _shard_w."""
    perm = _feat_perm(HID)
    wt = wmat.T[perm.reshape(-1)].reshape(KH, KP,
                                          wmat.shape[0]).astype(np.float32)
    o8p = W_ * nj
    c_idx = np.arange(o8p)
    f = (c_idx % W_) * nj + c_idx // W_
    valid = f < o8
    out = []
    for c in range(NCORES):
        blk = wt[:, :, c * o8:(c + 1) * o8]
        padded = np.zeros((KH, KP, o8p), np.float32)
        padded[:, :, valid] = blk[:, :, f[valid]]
        w8 = np.zeros((KP, KHP, 2, OQ8), NP8)
        for s in range(2):
            for cc in range(HID // 2):
                kk = s * (HID // 2) + cc
                k0 = s * HID + 2 * cc
                w8[:, kk, 0, :o8p] = padded[k0].astype(NP8)
                w8[:, kk, 1, :o8p] = padded[k0 + 1].astype(NP8)
        out.append(np.ascontiguousarray(w8))
    return out


def _pad_bias(bvec, o8, W_, nj):
    out = []
    p = np.arange(W_)
    for c in range(NCORES):
        bp = np.zeros((nj, 128), np.float32)
        for j in range(nj):
            f = p * nj + j
            m = f < o8
            bp[j, p[m]] = bvec[c * o8 + f[m]]
        out.append(bp)
    return out


def prep_inputs(inputs):
    x = np.asarray(inputs["x"], np.float32)
    permx = _feat_perm(C)
    xt = np.zeros((T, KP, KX, B), NPBF)
    for t in range(T):
        flat = x[:, t].reshape(B, C * 196).T      # [588, B]
        xt[t] = flat[permx.reshape(-1)].reshape(KX, KP, B).transpose(1, 0, 2)

    # td outputs are reloaded straight into device channel order [h, bu],
    # l-major interleaved across cores (device slot l*8+c = core c's l-th
    # channel, so each reload DMA covers all 8 cores): core c's td block is
    # [torch h {c, 8+c, 16+c, 24+c} | torch bu {2c, 2c+1}]. The h side is
    # the identity on device partitions; the bu side lands at slot
    # q=lb*8+c holding torch bu 2c+lb.
    ci_out = np.zeros(CIN, np.int64)
    for d in range(CIN):
        c, l = d // 6, d % 6
        ci_out[d] = (IND + l * 8 + c) if l < 4 else (2 * c + l - 4)
    o_perm = (ci_out[:, None] * 196 + np.arange(196)[None, :]).reshape(-1)
    tw0 = _shard_w8(np.asarray(inputs["td_w0"], np.float32)[o_perm],
                    OTD8, WTD, NJ_TD)
    tw1 = _shard_w8(np.asarray(inputs["td_w1"], np.float32)[o_perm],
                    OTD8, WTD, NJ_TD)
    bw0 = _shard_w(np.asarray(inputs["bu_w0"], np.float32), C, OBU8, WBU,
                   NJ_BU)
    bw1 = _shard_w(np.asarray(inputs["bu_w1"], np.float32), HID, OBU8, WBU,
                   NJ_BU)
    bw2 = _shard_w(np.asarray(inputs["bu_w2"], np.float32), HID, OBU8, WBU,
                   NJ_BU)
    tb0 = _pad_bias(np.asarray(inputs["td_b0"], np.float32)[o_perm], OTD8,
                    WTD, NJ_TD)
    tb1 = _pad_bias(np.asarray(inputs["td_b1"], np.float32)[o_perm], OTD8,
                    WTD, NJ_TD)
    bb0 = _pad_bias(np.asarray(inputs["bu_b0"], np.float32), OBU8, WBU, NJ_BU)
    bb1 = _pad_bias(np.asarray(inputs["bu_b1"], np.float32), OBU8, WBU, NJ_BU)
    bb2 = _pad_bias(np.asarray(inputs["bu_b2"], np.float32), OBU8, WBU, NJ_BU)

    # conv weights: device ci order = [h (0:32) -> torch ci 16..47, bu -> 0..15]
    # packed for 2-tap passes: wg2[n, iy] rows 0:48 = tap (dy,-1),
    # rows 48:96 = tap (dy,0); wg1[n, iy] = tap (dy,+1).
    # conv input rows: [h natural | bu l-major: device slot q holds torch
    # bu channel 2*(q%8) + q//8], matching the l-major reload layout
    pi16 = 2 * (np.arange(IND) % 8) + np.arange(IND) // 8
    ci_perm = np.concatenate([np.arange(IND, CIN), pi16])
    Wg = np.asarray(inputs["Wg"], np.float32)
    Wc = np.asarray(inputs["Wc"], np.float32)
    wg2 = np.zeros((N, 3, KPAIR, 2 * HID), NPBF)
    wg1 = np.zeros((N, 3, CIN, 2 * HID), NPBF)
    wc2 = np.zeros((N, 3, KPAIR, HID), NPBF)
    wc1 = np.zeros((N, 3, CIN, HID), NPBF)
    for iy in range(3):
        for n in range(N):
            wg2[n, iy, 0:CIN] = Wg[n][:, ci_perm, iy, 0].T
            wg2[n, iy, 64:] = Wg[n][:, ci_perm, iy, 1].T
            wg1[n, iy] = Wg[n][:, ci_perm, iy, 2].T
            wc2[n, iy, 0:CIN] = Wc[n][:, ci_perm, iy, 0].T
            wc2[n, iy, 64:] = Wc[n][:, ci_perm, iy, 1].T
            wc1[n, iy] = Wc[n][:, ci_perm, iy, 2].T

    permh = _feat_perm(HID)
    fc1 = np.asarray(inputs["fc1_w"], np.float32)     # (100, 6272)
    fc1p = np.zeros((NCORES * OFC, fc1.shape[1]), np.float32)
    fc1p[0:100] = fc1
    # [6272, 104] -> perm -> [KH, KP, 104] -> [KP, KH, 104] bf16, col-sharded
    fc1t_full = fc1p.T[permh.reshape(-1)].reshape(KH, KP, NCORES * OFC)
    fc1t_full = fc1t_full.transpose(1, 0, 2).astype(NPBF)
    fc1t = [np.ascontiguousarray(fc1t_full[:, :, c * OFC:(c + 1) * OFC])
            for c in range(NCORES)]
    fc2t = np.ascontiguousarray(np.asarray(inputs["fc2_w"], np.float32).T)  # (100, 10)

    common = {
        "xt": xt,
        "wg2": wg2, "wg1": wg1, "wc2": wc2, "wc1": wc1,
        "bg": np.asarray(inputs["bg"], np.float32),
        "bc": np.asarray(inputs["bc"], np.float32),
        "fc1b": np.asarray(inputs["fc1_b"], np.float32).reshape(100, 1),
        "fc2t": fc2t,
        "fc2b": np.asarray(inputs["fc2_b"], np.float32).reshape(10, 1),
        "ident": np.eye(32, dtype=np.float32).astype(NPBF),
    }
    in_maps = []
    for c in range(NCORES):
        m = dict(common)
        m.update({
            "tw0": tw0[c], "tw1": tw1[c], "bw0": bw0[c],
            "bw1": bw1[c], "bw2": bw2[c],
            "tb0": tb0[c], "tb1": tb1[c],
            "bb0": bb0[c], "bb1": bb1[c],
            "bb2": bb2[c], "fc1t": fc1t[c],
        })
        in_maps.append(m)
    return in_maps


def get_graph():
    if "nc" not in _CACHED:
        _CACHED["nc"] = build_graph()
    return _CACHED["nc"]


def kernel(**inputs):
    nc = get_graph()
    in_maps = prep_inputs(inputs)
    res = bass_utils.run_bass_kernel_spmd(nc, in_maps, core_ids=list(range(NCORES)))
    out_t = np.asarray(res.results[0]["out"]).reshape(10, B)
    return np.ascontiguousarray(out_t.T).astype(np.float32)

